# revision 1
# baseline (speedup 1.0000x reference)
"""Trainium2 Bass kernel for nn_DistillationLoss (normalize + shifted softmax +
top-50 column selection + per-sample Sinkhorn loss).

Strategy (8 NeuronCores, one SPMD program):
  - Core c < 4 processes student sample b=c; core c >= 4 teacher sample b=c-4.
  - Each core receives only the answer-span rows of its sample's logits
    [512, 32000] (plus a host-pretransposed copy [32000, 512] used for the
    dynamic top-50 column gather) -> phase 1 reads 65.5 MB/core, the memory
    floor for this problem.
  - Phase 1 (streamed): per-row mean/var (bn_stats), rstd, exp(x*rstd) with
    fused row-sum accumulation on the scalar engine, softmax-weighted column
    sums via TensorE transpose-matmuls into PSUM.
  - One 8-core AllReduce combines the student/teacher column sums; every core
    redundantly extracts both ordered top-50 index lists (max8/match_replace
    tournament), picks its own role's list, and gathers the 50 selected vocab
    rows from the transposed logits with one indirect DMA.
  - Pairwise AllGather {b, b+4} exchanges the selected softmax probabilities;
    both pair members then compute the same per-sample Sinkhorn loss (|d| on
    the scalar engine via Abs(x + bias), accumulation on vector engine, 10
    row/col normalization iterations).
  - Host sums the 4 student-core partials with the CE term.
"""
import numpy as np

import concourse.bass as bass
import concourse.bacc as bacc
import concourse.tile as tile
from concourse import mybir
from concourse.bass_utils import run_bass_kernel_spmd
from concourse.masks import make_identity

F32 = mybir.dt.float32
I32 = mybir.dt.int32
U32 = mybir.dt.uint32
AX = mybir.AxisListType
OP = mybir.AluOpType
ACTF = mybir.ActivationFunctionType

N_CORES = 8
B, T, V_FULL, ANS = 4, 1024, 32000, 512
K = 50
NCAND_R = 7          # rounds of top-8 -> 56 >= 50 candidates
NCAND = NCAND_R * 8  # 56
IGNORE = -100


def _bn_pieces(fc):
    """Equal-width pieces <= 512, multiples of 128 (bn_aggr mis-weights
    unequal chunk counts; colsum relayout needs 128-aligned pieces)."""
    np_ = -(-fc // 512)
    while fc % np_ or (fc // np_) % 128:
        np_ += 1
    w = fc // np_
    return [(i * w, w) for i in range(np_)]


def build(L=ANS, V=V_FULL, FC=3200, n_cores=N_CORES, chunk_bufs=14,
          stop_after=None):
    assert L % 128 == 0 and V % FC == 0 and FC % 128 == 0
    RT = L // 128          # row tiles
    NCH = V // FC          # dma chunks per row tile
    SUB = FC // 128        # 128-wide matmul subchunks per dma chunk
    COLS = V // 128        # colsum free size (vocab v = f*128 + p)
    pieces = _bn_pieces(FC)
    NP = len(pieces)

    order = ["p1", "ar", "top50", "p2", "p3t", "p3w", "p3i", "p3"]
    stage = len(order) if stop_after is None else order.index(stop_after) + 1

    nc = bacc.Bacc("TRN2", target_bir_lowering=False, debug=False,
                   num_devices=n_cores)
    xs = nc.dram_tensor("xs", [L, V], F32, kind="ExternalInput").ap()
    xsT = nc.dram_tensor("xsT", [V, L], F32, kind="ExternalInput").ap()
    colsel = nc.dram_tensor("colsel", [128, 2], F32, kind="ExternalInput").ap()
    partial = nc.dram_tensor("partial", [1, 1], F32, kind="ExternalOutput").ap()
    dbg_sel = nc.dram_tensor("dbg_sel", [NCAND, 2], F32,
                             kind="ExternalOutput").ap()

    with tile.TileContext(nc) as tc:
        with tc.tile_pool(name="const", bufs=1) as cp, \
             tc.tile_pool(name="dram", bufs=1, space="DRAM") as dram:
            ident = cp.tile([128, 128], F32)
            make_identity(nc, ident[:])
            ones_row = cp.tile([1, 128], F32)
            nc.vector.memset(ones_row[:], 1.0)
            ones_col = cp.tile([128, 1], F32)
            nc.vector.memset(ones_col[:], 1.0)
            pidx_i = cp.tile([128, 1], I32)
            nc.gpsimd.iota(pidx_i[:], pattern=[[0, 1]], base=0,
                           channel_multiplier=1)
            pidx = cp.tile([128, 1], F32)
            nc.vector.tensor_copy(pidx[:], pidx_i[:])
            cs_t = cp.tile([128, 2], F32)
            nc.sync.dma_start(cs_t[:], colsel[:])
            rstd_keep = cp.tile([128, RT], F32)
            rden_keep = cp.tile([128, RT], F32)
            rstd_row = cp.tile([1, L], F32)
            rden_row = cp.tile([1, L], F32)
            colacc = cp.tile([128, COLS], F32)
            csr = []
            for rt in range(RT):
                csr_rt = cp.tile([128, COLS], F32, tag=f"csr{rt}")
                csr.append(csr_rt)
            sel_i = cp.tile([NCAND, 1], I32)

            # ---------------- phase 1: stats + exp + column sums ----------
            with tc.tile_pool(name="p1", bufs=chunk_bufs) as p1, \
                 tc.tile_pool(name="p1aux", bufs=2) as p1a, \
                 tc.tile_pool(name="p1ps", bufs=2, space="PSUM") as p1ps:
                for rt in range(RT):
                    rsl = slice(rt * 128, (rt + 1) * 128)
                    stats = p1a.tile([128, NCH * NP, 6], F32, tag="stats")
                    esum = p1a.tile([128, NCH], F32, tag="esum")
                    chunks = []
                    for c in range(NCH):
                        ch = p1.tile([128, FC], F32, tag="xchunk")
                        nc.sync.dma_start(ch[:], xs[rsl, c * FC:(c + 1) * FC])
                        chunks.append(ch)
                        for pi, (off, w) in enumerate(pieces):
                            nc.vector.bn_stats(stats[:, c * NP + pi],
                                               ch[:, off:off + w])
                    mv = p1a.tile([128, 2], F32, tag="mv")
                    nc.vector.bn_aggr(mv[:], stats[:])
                    # rstd = 1 / max(sqrt(var * V/(V-1)), 1e-6)
                    vs = p1a.tile([128, 1], F32, tag="vs")
                    nc.vector.tensor_scalar(
                        out=vs[:], in0=mv[:, 1:2], scalar1=float(V) / (V - 1),
                        scalar2=None, op0=OP.mult)
                    nc.scalar.sqrt(vs[:], vs[:])
                    nc.vector.tensor_scalar(
                        out=vs[:], in0=vs[:], scalar1=1e-6, scalar2=None,
                        op0=OP.max)
                    nc.vector.reciprocal(rstd_keep[:, rt:rt + 1], vs[:])
                    nc.sync.dma_start(rstd_row[0:1, rt * 128:(rt + 1) * 128],
                                      rstd_keep[:, rt:rt + 1])
                    for c in range(NCH):
                        nc.scalar.activation(
                            chunks[c][:], chunks[c][:], ACTF.Exp,
                            scale=rstd_keep[:, rt:rt + 1],
                            accum_out=esum[:, c:c + 1])
                    tot = p1a.tile([128, 1], F32, tag="tot")
                    nc.vector.reduce_sum(tot[:], esum[:], axis=AX.X)
                    nc.vector.reciprocal(rden_keep[:, rt:rt + 1], tot[:])
                    nc.sync.dma_start(rden_row[0:1, rt * 128:(rt + 1) * 128],
                                      rden_keep[:, rt:rt + 1])
                    # weighted column sums via transpose-matmuls:
                    # stationary = exp subchunk [128r, 128v], moving = rdenom
                    # -> psum[128v, 1] at column j (vocab v = j*128 + p)
                    cs = p1ps.tile([128, COLS], F32, tag="cs")
                    for c in range(NCH):
                        for sb in range(SUB):
                            j = c * SUB + sb
                            nc.tensor.matmul(
                                cs[:, j:j + 1],
                                lhsT=chunks[c][:, sb * 128:(sb + 1) * 128],
                                rhs=rden_keep[:, rt:rt + 1],
                                start=True, stop=True)
                    nc.vector.tensor_copy(csr[rt][:], cs[:])

            with tc.tile_pool(name="p1sum", bufs=1) as p1s:
                t01 = p1s.tile([128, COLS], F32, tag="t01")
                nc.vector.tensor_tensor(out=t01[:], in0=csr[0][:],
                                        in1=csr[1][:], op=OP.add)
                t23 = p1s.tile([128, COLS], F32, tag="t23")
                nc.vector.tensor_tensor(out=t23[:], in0=csr[2][:],
                                        in1=csr[3][:], op=OP.add)
                nc.vector.tensor_tensor(out=colacc[:], in0=t01[:],
                                        in1=t23[:], op=OP.add)

            if stage >= 2:
                # ---------------- colsum allreduce -------------------------
                ar_in = dram.tile([2, 128, COLS], F32)
                ar_out = dram.tile([2, 128, COLS], F32)
                gs = cp.tile([128, 2, COLS], F32)
                with tc.tile_pool(name="p15a", bufs=1) as p15a:
                    half0 = p15a.tile([128, COLS], F32, tag="half0")
                    half1 = p15a.tile([128, COLS], F32, tag="half1")
                    nc.vector.tensor_scalar(out=half0[:], in0=colacc[:],
                                            scalar1=cs_t[:, 0:1], scalar2=None,
                                            op0=OP.mult)
                    nc.vector.tensor_scalar(out=half1[:], in0=colacc[:],
                                            scalar1=cs_t[:, 1:2], scalar2=None,
                                            op0=OP.mult)
                    nc.sync.dma_start(ar_in[0], half0[:])
                    nc.sync.dma_start(ar_in[1], half1[:])
                    nc.gpsimd.collective_compute(
                        "AllReduce", OP.add,
                        replica_groups=[list(range(n_cores))],
                        ins=[ar_in.opt()], outs=[ar_out.opt()])
                    nc.sync.dma_start(gs[:],
                                      ar_out[:].rearrange("h p f -> p h f"))

            if stage >= 3:
                # ---------------- ordered top-50 per half ------------------
                # hierarchy: [128,COLS] -> cand [128,56] -> [16,448]
                # -> cand [16,56] -> [1,896] -> top-56 in order
                with tc.tile_pool(name="p15", bufs=1) as p15:
                    g448 = p15.tile([16, 1], F32, tag="g448")
                    gi = p15.tile([16, 1], I32, tag="gi")
                    nc.gpsimd.iota(gi[:], pattern=[[0, 1]], base=0,
                                   channel_multiplier=448)
                    nc.vector.tensor_copy(g448[:], gi[:])
                    sel_rows = []
                    for h in range(2):
                        work = p15.tile([128, COLS], F32, tag=f"work{h}")
                        nc.vector.tensor_copy(work[:], gs[:, h])
                        cvals = p15.tile([128, NCAND], F32, tag=f"cvals{h}")
                        cpos = p15.tile([128, NCAND], U32, tag=f"cpos{h}")
                        for r in range(NCAND_R):
                            m8 = p15.tile([128, 8], F32, tag=f"m8{h}")
                            nc.vector.max(m8[:], work[:])
                            nc.vector.max_index(cpos[:, r * 8:(r + 1) * 8],
                                                m8[:], work[:])
                            nc.vector.tensor_copy(
                                cvals[:, r * 8:(r + 1) * 8], m8[:])
                            nc.vector.match_replace(work[:], m8[:], work[:],
                                                    -1e30)
                        cposf = p15.tile([128, NCAND], F32, tag=f"cposf{h}")
                        nc.vector.tensor_copy(cposf[:], cpos[:])
                        cvid = p15.tile([128, NCAND], F32, tag=f"cvid{h}")
                        # vocab id = pos*128 + p
                        nc.vector.tensor_scalar(
                            out=cvid[:], in0=cposf[:], scalar1=128.0,
                            scalar2=pidx[:, 0:1], op0=OP.mult, op1=OP.add)
                        # flat candidate order i = p*NCAND + j
                        vid_dram = dram.tile([128 * NCAND, 1], F32,
                                             tag=f"vid_dram{h}")
                        l2v = p15.tile([16, 8 * NCAND], F32, tag=f"l2v{h}")
                        nc.sync.dma_start(l2v[:], cvals[:])
                        l2vid = p15.tile([16, 8 * NCAND], F32,
                                         tag=f"l2vid{h}")
                        nc.sync.dma_start(l2vid[:], cvid[:])
                        nc.sync.dma_start(
                            vid_dram[:].rearrange("(g n) w -> g (n w)", g=16),
                            l2vid[:])
                        # level 2: per-partition top-56 of [16, 448]
                        l2cv = p15.tile([16, NCAND], F32, tag=f"l2cv{h}")
                        l2cp = p15.tile([16, NCAND], U32, tag=f"l2cp{h}")
                        for r in range(NCAND_R):
                            n8 = p15.tile([16, 8], F32, tag=f"n8{h}")
                            nc.vector.max(n8[:], l2v[:])
                            nc.vector.max_index(l2cp[:, r * 8:(r + 1) * 8],
                                                n8[:], l2v[:])
                            nc.vector.tensor_copy(
                                l2cv[:, r * 8:(r + 1) * 8], n8[:])
                            nc.vector.match_replace(l2v[:], n8[:], l2v[:],
                                                    -1e30)
                        l2cpf = p15.tile([16, NCAND], F32, tag=f"l2cpf{h}")
                        nc.vector.tensor_copy(l2cpf[:], l2cp[:])
                        l2flat = p15.tile([16, NCAND], F32, tag=f"l2flat{h}")
                        # flat-7168 position = g*448 + pos448
                        nc.vector.tensor_scalar(
                            out=l2flat[:], in0=l2cpf[:], scalar1=1.0,
                            scalar2=g448[:, 0:1], op0=OP.mult, op1=OP.add)
                        flat_dram = dram.tile([16 * NCAND, 1], F32,
                                              tag=f"flat_dram{h}")
                        l3v = p15.tile([1, 16 * NCAND], F32, tag=f"l3v{h}")
                        nc.sync.dma_start(l3v[0:1, :], l2cv[:])
                        l3f = p15.tile([1, 16 * NCAND], F32, tag=f"l3f{h}")
                        nc.sync.dma_start(l3f[0:1, :], l2flat[:])
                        nc.sync.dma_start(
                            flat_dram[:].rearrange("(o n) w -> o (n w)", o=1),
                            l3f[0:1, :])
                        # level 3: global ordered top-56 of [1, 896]
                        gpos = p15.tile([1, NCAND], U32, tag=f"gpos{h}")
                        for r in range(NCAND_R):
                            g8 = p15.tile([1, 8], F32, tag=f"g8{h}")
                            nc.vector.max(g8[:], l3v[:])
                            nc.vector.max_index(gpos[:, r * 8:(r + 1) * 8],
                                                g8[:], l3v[:])
                            nc.vector.match_replace(l3v[:], g8[:], l3v[:],
                                                    -1e30)
                        gpos_col = p15.tile([NCAND, 1], U32, tag=f"gposc{h}")
                        nc.sync.dma_start(gpos_col[:, 0:1], gpos[0:1, :])
                        # chase: 896-pos -> flat-7168 pos -> vocab id
                        fpos = p15.tile([NCAND, 1], F32, tag=f"fpos{h}")
                        nc.gpsimd.indirect_dma_start(
                            out=fpos[:], out_offset=None,
                            in_=flat_dram[:],
                            in_offset=bass.IndirectOffsetOnAxis(
                                ap=gpos_col[:, 0:1].bitcast(I32), axis=0))
                        fpos_i = p15.tile([NCAND, 1], I32, tag=f"fposi{h}")
                        nc.vector.tensor_copy(fpos_i[:], fpos[:])
                        svid = p15.tile([NCAND, 1], F32, tag=f"svid{h}")
                        nc.gpsimd.indirect_dma_start(
                            out=svid[:], out_offset=None,
                            in_=vid_dram[:],
                            in_offset=bass.IndirectOffsetOnAxis(
                                ap=fpos_i[:, 0:1], axis=0))
                        nc.sync.dma_start(dbg_sel[:, h:h + 1], svid[:])
                        sel_rows.append(svid)

                    # blend by role and convert to int indices
                    selb = p15.tile([NCAND, 1], F32, tag="selb")
                    nc.vector.tensor_scalar(
                        out=selb[:], in0=sel_rows[0][:],
                        scalar1=cs_t[0:NCAND, 0:1], scalar2=None, op0=OP.mult)
                    nc.vector.scalar_tensor_tensor(
                        out=selb[:], in0=sel_rows[1][:],
                        scalar=cs_t[0:NCAND, 1:2], in1=selb[:],
                        op0=OP.mult, op1=OP.add)
                    nc.vector.tensor_copy(sel_i[:], selb[:])

            if stage >= 4:
                # ------------- phase 2: gather + selected softmax ----------
                ag_in = dram.tile([K, L], F32)
                ag_out = dram.tile([2, K, L], F32)
                with tc.tile_pool(name="p2", bufs=1) as p2, \
                     tc.tile_pool(name="p2ps", bufs=2, space="PSUM") as p2ps:
                    xsel = p2.tile([K, L], F32, tag="xsel")
                    nc.gpsimd.indirect_dma_start(
                        out=xsel[:], out_offset=None, in_=xsT[:],
                        in_offset=bass.IndirectOffsetOnAxis(
                            ap=sel_i[0:K, 0:1], axis=0))
                    rep_std = p2ps.tile([128, L], F32, tag="rep")
                    nc.tensor.matmul(rep_std[:], lhsT=ones_row[:],
                                     rhs=rstd_row[:], start=True, stop=True)
                    xn = p2.tile([K, L], F32, tag="xn")
                    nc.vector.tensor_tensor(out=xn[:], in0=xsel[:],
                                            in1=rep_std[0:K, :], op=OP.mult)
                    es = p2.tile([K, L], F32, tag="es")
                    nc.scalar.activation(es[:], xn[:], ACTF.Exp)
                    rep_den = p2ps.tile([128, L], F32, tag="rep")
                    nc.tensor.matmul(rep_den[:], lhsT=ones_row[:],
                                     rhs=rden_row[:], start=True, stop=True)
                    psel = p2.tile([K, L], F32, tag="psel")
                    nc.vector.tensor_tensor(out=psel[:], in0=es[:],
                                            in1=rep_den[0:K, :], op=OP.mult)
                    # sinkhorn-seq softmax (T=2) over the K dim (partitions)
                    es2 = p2.tile([K, L], F32, tag="es2")
                    nc.scalar.activation(es2[:], psel[:], ACTF.Exp, scale=0.5)
                    den2 = p2ps.tile([1, L], F32, tag="den2")
                    nc.tensor.matmul(den2[:], lhsT=ones_col[0:K, 0:1],
                                     rhs=es2[:], start=True, stop=True)
                    rec2 = p2.tile([1, L], F32, tag="rec2")
                    nc.vector.reciprocal(rec2[:], den2[:])
                    rep2 = p2ps.tile([128, L], F32, tag="rep")
                    nc.tensor.matmul(rep2[:], lhsT=ones_row[:], rhs=rec2[:],
                                     start=True, stop=True)
                    my_pT = p2.tile([K, L], F32, tag="my_pT")
                    nc.vector.tensor_tensor(out=my_pT[:], in0=es2[:],
                                            in1=rep2[0:K, :], op=OP.mult)
                    nc.sync.dma_start(ag_in[:], my_pT[:])
                    nc.gpsimd.collective_compute(
                        "AllGather", OP.bypass,
                        replica_groups=[[b, b + 4] for b in range(4)],
                        ins=[ag_in.opt()], outs=[ag_out.opt()])

            if stage >= 5:
                # ------------- phase 3: W + sinkhorn iterations ------------
                with tc.tile_pool(name="p3", bufs=1) as p3, \
                     tc.tile_pool(name="p3tmp", bufs=3) as p3t, \
                     tc.tile_pool(name="p3ps", bufs=1, space="PSUM") as p3ps, \
                     tc.tile_pool(name="p3ps2", bufs=2, space="PSUM") as p3ps2:
                    ps_T = p3.tile([K, L], F32, tag="ps_T")
                    nc.sync.dma_start(ps_T[:], ag_out[0])
                    pt_T = p3.tile([K, L], F32, tag="pt_T")
                    nc.sync.dma_start(pt_T[:], ag_out[1])
                    ptl = p3.tile([1, K * L], F32, tag="ptl")
                    nc.sync.dma_start(ptl[0:1, :], pt_T[:])
                    # transpose ps_T -> ps tiles [128, RT, K]
                    ps_i = p3.tile([128, RT, K], F32, tag="ps_i")
                    for rt in range(RT):
                        trp = p3ps.tile([128, K], F32, tag="trp")
                        nc.tensor.transpose(trp[:],
                                            ps_T[:, rt * 128:(rt + 1) * 128],
                                            ident[0:K, 0:K])
                        nc.vector.tensor_copy(ps_i[:, rt], trp[:])

                    if stage < 6:
                        plast3 = p3t.tile([1, 1], F32, tag="plast3")
                        nc.vector.tensor_copy(plast3[:], ps_i[0:1, 0, 0:1])
                        nc.sync.dma_start(partial[:], plast3[:])
                    ps_neg = p3.tile([128, RT, K], F32, tag="ps_neg")
                    nc.vector.tensor_scalar(out=ps_neg[:], in0=ps_i[:],
                                            scalar1=-1.0, scalar2=None,
                                            op0=OP.mult)
                    Wm = p3.tile([128, RT, L], F32, tag="Wm")
                    nc.vector.memset(Wm[:], 0.0)
                    for k in range(K if stage >= 6 else 0):
                        repk = p3ps2.tile([128, L], F32, tag="repk")
                        nc.tensor.matmul(repk[:], lhsT=ones_row[:],
                                         rhs=ptl[0:1, k * L:(k + 1) * L],
                                         start=True, stop=True)
                        for rt in range(RT):
                            tmp = p3t.tile([128, L], F32, tag="wtmp")
                            # |pt[j,k] - ps[i,k]| on ScalarE
                            nc.scalar.activation(
                                tmp[:], repk[:], ACTF.Abs,
                                bias=ps_neg[:, rt, k:k + 1])
                            nc.vector.tensor_tensor(out=Wm[:, rt],
                                                    in0=Wm[:, rt],
                                                    in1=tmp[:], op=OP.add)

                    if stage == 6:
                        plast4 = p3t.tile([1, 1], F32, tag="plast4")
                        nc.vector.tensor_copy(plast4[:], Wm[0:1, 0, 0:1])
                        nc.sync.dma_start(partial[:], plast4[:])
                    Pm = p3.tile([128, RT, L], F32, tag="Pm")
                    for rt in range(RT):
                        nc.scalar.activation(Pm[:, rt], Wm[:, rt], ACTF.Exp,
                                             scale=-2.0)
                    rs = p3t.tile([128, RT], F32, tag="rs")
                    rr = p3t.tile([128, RT], F32, tag="rr")
                    for it in range(10 if stage >= 7 else 0):
                        for rt in range(RT):
                            nc.vector.reduce_sum(rs[:, rt:rt + 1], Pm[:, rt],
                                                 axis=AX.X)
                        nc.vector.reciprocal(rr[:], rs[:])
                        # row-scale folded into the colsum matmul weights
                        csum = p3ps.tile([1, L], F32, tag="csum")
                        for rt in range(RT):
                            nc.tensor.matmul(csum[:],
                                             lhsT=rr[:, rt:rt + 1],
                                             rhs=Pm[:, rt], start=(rt == 0),
                                             stop=(rt == RT - 1))
                        crec = p3t.tile([1, L], F32, tag="crec")
                        nc.vector.reciprocal(crec[:], csum[:])
                        crep = p3ps2.tile([128, L], F32, tag="crep")
                        nc.tensor.matmul(crep[:], lhsT=ones_row[:],
                                         rhs=crec[:], start=True, stop=True)
                        for rt in range(RT):
                            nc.vector.scalar_tensor_tensor(
                                out=Pm[:, rt], in0=Pm[:, rt],
                                scalar=rr[:, rt:rt + 1], in1=crep[:],
                                op0=OP.mult, op1=OP.mult)

                    if stage == 7:
                        plast5 = p3t.tile([1, 1], F32, tag="plast5")
                        nc.vector.tensor_copy(plast5[:], Pm[0:1, 0, 0:1])
                        nc.sync.dma_start(partial[:], plast5[:])
                    ls = p3t.tile([128, RT], F32, tag="ls")
                    for rt in range(RT if stage >= 8 else 0):
                        scr = p3t.tile([128, L], F32, tag="scr")
                        nc.vector.tensor_tensor(out=scr[:], in0=Pm[:, rt],
                                                in1=Wm[:, rt], op=OP.mult)
                        nc.vector.reduce_sum(ls[:, rt:rt + 1], scr[:],
                                             axis=AX.X)
                    if stage < 8:
                        nc.vector.memset(ls[:], 0.0)
                    lsum = p3t.tile([128, 1], F32, tag="lsum")
                    nc.vector.reduce_sum(lsum[:], ls[:], axis=AX.X)
                    lps = p3ps.tile([1, 1], F32, tag="lps")
                    nc.tensor.matmul(lps[:], lhsT=ones_col[:], rhs=lsum[:],
                                     start=True, stop=True)
                    if stage >= 8:
                        lout = p3t.tile([1, 1], F32, tag="lout")
                        nc.vector.tensor_copy(lout[:], lps[:])
                        nc.sync.dma_start(partial[:], lout[:])

            if stage < 5:
                # bisect builds: emit something deterministic into partial
                with tc.tile_pool(name="pout", bufs=1) as po:
                    plast = po.tile([1, 1], F32)
                    nc.vector.tensor_copy(plast[:], colacc[0:1, 0:1])
                    nc.sync.dma_start(partial[:], plast[:])

    nc.compile()
    return nc


_NC_CACHE = {}


def _get_nc(key=(ANS, V_FULL, 3200), stop_after=None):
    ck = (key, stop_after)
    if ck not in _NC_CACHE:
        _NC_CACHE[ck] = build(L=key[0], V=key[1], FC=key[2],
                              stop_after=stop_after)
    return _NC_CACHE[ck]


def _prep_inputs(student_logits, teacher_logits, student_targets,
                 teacher_targets, mex_length):
    L = int(mex_length)
    assert L == ANS, f"kernel compiled for mex_length={ANS}, got {L}"
    s_mask = student_targets != IGNORE
    t_mask = teacher_targets != IGNORE
    s_start = np.argmax(s_mask, axis=1)
    t_start = np.argmax(t_mask, axis=1)
    assert (s_mask.sum(1) >= L).all() and (t_mask.sum(1) >= L).all(), \
        "kernel requires answer span >= mex_length"
    in_maps = []
    for c in range(N_CORES):
        b = c % 4
        if c < 4:
            st = int(s_start[b])
            shard = np.ascontiguousarray(student_logits[b, st:st + L, :])
            sel = np.array([1.0, 0.0], np.float32)
        else:
            st = int(t_start[b])
            shard = np.ascontiguousarray(teacher_logits[b, st:st + L, :])
            sel = np.array([0.0, 1.0], np.float32)
        in_maps.append({
            "xs": shard,
            "xsT": np.ascontiguousarray(shard.T),
            "colsel": np.broadcast_to(sel, (128, 2)).copy(),
        })
    return in_maps


def _run(inputs, trace=False, tmpdir=None, stop_after=None):
    nc = _get_nc(stop_after=stop_after)
    in_maps = _prep_inputs(
        inputs["student_logits"], inputs["teacher_logits"],
        inputs["student_targets"], inputs["teacher_targets"],
        inputs["mex_length"])
    res = run_bass_kernel_spmd(nc, in_maps, list(range(N_CORES)),
                               trace=trace, tmpdir=tmpdir)
    partials = [float(res.results[c]["partial"][0, 0]) for c in range(4)]
    emd = np.float32(0.001) * np.float32(np.sum(np.asarray(partials,
                                                           np.float32)))
    total = np.float32(inputs["student_ce_loss"][0]) + np.float32(emd)
    return np.asarray(total, np.float32), res


def kernel(**inputs) -> np.ndarray:
    out, _ = _run(inputs, trace=False)
    return out

